# revision 2
# baseline (speedup 1.0000x reference)
"""EPMoE (top-2, 16 experts) forward on 8 Trainium2 NeuronCores.

Strategy (expert parallel, hybrid dataflow):
  - Host: router softmax/top-2/renorm, token->expert dispatch (stable order,
    matching the reference), weight re-layout into slab-contiguous form,
    final weighted combine.
  - Device (per core, 2 experts): one LIGHT expert (<=128 tokens) computed
    with the token tile as the matmul stationary operand (weights stream as
    the moving operand, cutting LDWEIGHTS overhead ~10x and producing
    token-major outputs), and one HEAVY expert (<=152 tokens) computed with
    the classic weight-stationary grouped GEMM.  Tensor-engine phases are
    ordered L-G1 -> H-G1 -> L-G2 -> H-G2 so activation drains (silu / mul /
    act transpose) always hide under another expert's matmuls, and the
    weight DMA stream (the memory-roofline term: ~34.6 MB/core in bf16)
    matches consumption order exactly.

The reference's simulated fp8 quantization (amax scaling + clip, no rounding)
cancels exactly: (x/sa) @ (w/sw)^T * sa*sw == x @ w^T, and the +-448 clip
never binds for amax-scaled values.  So the kernel computes the plain MoE
forward in bf16 (real fp8 rounding would blow the 2e-2 error budget).
"""

import ml_dtypes
import numpy as np

import concourse.bass as bass
import concourse.bacc as bacc
import concourse.mybir as mybir
import concourse.tile as tile
from concourse.bass_utils import run_bass_kernel_spmd

dt = mybir.dt

# Problem shape (hardcoded per spec)
T, H, I, E, TOP_K = 1024, 2048, 1408, 16, 2
TWO_I = 2 * I
N_CORES = 8
EPC = E // N_CORES

KT1 = H // 128      # 16 contraction tiles for GEMM1
FT = I // 128       # 11 feature tiles per gate/up half
KT2 = I // 128      # 11 contraction tiles for GEMM2

# Light (X-stationary) expert: token count must be <= CL.
CL = 128
# GEMM1 f-columns are laid out as interleaved (gate, up) 512-chunks so each
# psum bank pair (g_j, u_j) drains to silu*mul independently.
#   pass A: [g 0:512 | u 0:512 | g 512:1024 | u 512:1024]   (2048 cols)
#   pass B: [g 1024:1408 | u 1024:1408]                     (768 cols)
LA_COLS = 2048
LB_COLS = 768
L2_CH = 4           # GEMM2 output 512-chunks (4 * 512 = 2048 = H)

# Heavy (W-stationary) expert: token count must be <= CH.
CH = 152
KB1 = 4             # k-tiles per heavy GEMM1 weight slab (one DMA)
MT_GRP = 2          # heavy GEMM2 m-groups
MT_G = H // 128 // MT_GRP  # 8 output tiles per m-group
MW = MT_G * 128     # 1024

_CACHED_NC = None
_CACHED_FB = {}

F32R = dt.float32r
BF16 = dt.bfloat16
SILU = mybir.ActivationFunctionType.Silu
COPY = mybir.ActivationFunctionType.Copy


def _build_program():
    """One SPMD program: per core, 1 light + 1 heavy expert MoE FFN.

    DRAM layouts (host pre-arranged, all bf16):
      xl   [128, KT1, CL]       x^T of the light expert's tokens
      xh   [128, KT1, CH]       x^T of the heavy expert's tokens
      w13la[KT1, 128, LA_COLS]  light w13^T cols [g0|u0|g1|u1] (512-chunks)
      w13lb[KT1, 128, LB_COLS]  light w13^T cols [g2|u2] (384-chunks)
      w2l  [KT2, 128, H]        light w2^T
      w13h [2, KT1//KB1, 128, KB1, I]  heavy w13^T, gate/up halves
      w2h  [MT_GRP, 128, KT2, MW]      heavy w2^T
      yl   [128, H]   (out)     light y, token-major
      yh   [MT_GRP, 128, MT_G, CH] (out)  heavy y^T
    """
    nc = bacc.Bacc("TRN2", target_bir_lowering=False, debug=False,
                   num_devices=N_CORES)

    xl = nc.declare_dram_parameter("xl", [128, KT1, CL], BF16, isOutput=False)
    xh = nc.declare_dram_parameter("xh", [128, KT1, CH], BF16, isOutput=False)
    w13la = nc.declare_dram_parameter("w13la", [KT1, 128, LA_COLS], BF16,
                                      isOutput=False)
    w13lb = nc.declare_dram_parameter("w13lb", [KT1, 128, LB_COLS], BF16,
                                      isOutput=False)
    w2l = nc.declare_dram_parameter("w2l", [KT2, 128, H], BF16, isOutput=False)
    w13h = nc.declare_dram_parameter("w13h", [2, KT1 // KB1, 128, KB1, I],
                                     BF16, isOutput=False)
    w2h = nc.declare_dram_parameter("w2h", [MT_GRP, 128, KT2, MW], BF16,
                                    isOutput=False)
    yl = nc.declare_dram_parameter("yl", [128, H], BF16, isOutput=True)
    yh = nc.declare_dram_parameter("yh", [MT_GRP, 128, MT_G, CH], BF16,
                                   isOutput=True)

    with tile.TileContext(nc) as tc:
        with (
            tc.tile_pool(name="pxl", bufs=1) as pxl,
            tc.tile_pool(name="pxh", bufs=1) as pxh,
            tc.tile_pool(name="pwla", bufs=5) as pwla,
            tc.tile_pool(name="pwlb", bufs=4) as pwlb,
            tc.tile_pool(name="pw13h", bufs=3) as pw13h,
            tc.tile_pool(name="pw2l", bufs=5) as pw2l,
            tc.tile_pool(name="pw2h", bufs=2) as pw2h,
            tc.tile_pool(name="psl", bufs=1) as psl,
            tc.tile_pool(name="pactl", bufs=1) as pactl,
            tc.tile_pool(name="pactlt", bufs=1) as pactlt,
            tc.tile_pool(name="psh", bufs=FT + 1) as psh,
            tc.tile_pool(name="pacth", bufs=1) as pacth,
            tc.tile_pool(name="pyl", bufs=1) as pyl,
            tc.tile_pool(name="pyh", bufs=2) as pyh,
            tc.tile_pool(name="psum", bufs=8, space="PSUM") as pp,
        ):
            # x loads on the scalar HWDGE queue so they run concurrently
            # with the weight stream on the sync queue from t=0.
            xlt = pxl.tile([128, KT1, CL], BF16, tag="xl")
            xht = pxh.tile([128, KT1, CH], BF16, tag="xh")
            nc.scalar.dma_start(xlt[:, :4, :], xl[:, :4, :])
            nc.scalar.dma_start(xlt[:, 4:, :], xl[:, 4:, :])
            nc.scalar.dma_start(xht[:], xh[:])

            # ---- light GEMM1 pass A: psum chunks [g0 | u0 | g1 | u1] ----
            psA = [pp.tile([128, 512], dt.float32, tag="ps", name=f"psA{i}")
                   for i in range(4)]
            for kh in range(KT1):
                slab = pwla.tile([128, LA_COLS], BF16, tag="wla")
                # finest pieces on the first slabs so the PE starts early
                np_pieces = 4 if kh == 0 else (2 if kh <= 2 else 1)
                step = LA_COLS // np_pieces
                for pi in range(np_pieces):
                    nc.sync.dma_start(slab[:, pi * step:(pi + 1) * step],
                                      w13la[kh, :, pi * step:(pi + 1) * step])
                for ch in range(4):
                    nc.tensor.matmul(
                        psA[ch][:], xlt[:, kh, :],
                        slab[:, ch * 512:(ch + 1) * 512],
                        start=(kh == 0), stop=(kh == KT1 - 1),
                    )
            # drain A while pass B computes: act[:, 0:1024]
            slt = psl.tile([128, I], BF16, tag="sl")
            actl = pactl.tile([128, I], BF16, tag="actl")
            for pr in range(2):
                nc.scalar.activation(slt[:, pr * 512:(pr + 1) * 512],
                                     psA[2 * pr][:], SILU)
                nc.vector.tensor_mul(actl[:, pr * 512:(pr + 1) * 512],
                                     slt[:, pr * 512:(pr + 1) * 512],
                                     psA[2 * pr + 1][:])

            # ---- light GEMM1 pass B: psum chunks [g2 | u2] (384 wide) ----
            psB = [pp.tile([128, 512], dt.float32, tag="ps", name=f"psB{i}")
                   for i in range(2)]
            for kh in range(KT1):
                slab = pwlb.tile([128, LB_COLS], BF16, tag="wlb")
                nc.sync.dma_start(slab[:], w13lb[kh, :, :])
                for ch in range(2):
                    nc.tensor.matmul(
                        psB[ch][:, :384], xlt[:, kh, :],
                        slab[:, ch * 384:(ch + 1) * 384],
                        start=(kh == 0), stop=(kh == KT1 - 1),
                    )
            nc.scalar.activation(slt[:, 1024:I], psB[0][:, :384], SILU)
            nc.vector.tensor_mul(actl[:, 1024:I], slt[:, 1024:I],
                                 psB[1][:, :384])

            # act transpose for light GEMM2 (xbar DMA, zero tensor cost)
            actlt = pactlt.tile([128, KT2, 128], BF16, tag="actlt")
            for j in range(KT2):
                nc.scalar.dma_start_transpose(
                    actlt[:, j, :], actl[:, j * 128:(j + 1) * 128])

            # ---- heavy GEMM1 (W-stationary, baseline structure) ----
            silu_tiles = []
            acth = pacth.tile([128, KT2, CH], BF16, tag="acth")
            for fh in range(2):  # 0 = gate half, 1 = up half
                pst = [pp.tile([128, 512], dt.float32, tag="ps",
                               name=f"psH{fh}_{i}")
                       for i in range((FT + 1) // 2)]
                for kh in range(KT1 // KB1):
                    slab = pw13h.tile([128, KB1, I], BF16, tag="w13h")
                    for pi in range(2):
                        lo = pi * (KB1 // 2)
                        nc.sync.dma_start(slab[:, lo:lo + KB1 // 2, :],
                                          w13h[fh, kh, :, lo:lo + KB1 // 2, :])
                    for kk in range(KB1):
                        k = kh * KB1 + kk
                        for j in range(FT):
                            dst = pst[j // 2][:,
                                              (j % 2) * CH:(j % 2 + 1) * CH]
                            # start=True clears has_written for the WHOLE
                            # bank: only the first group packed into each
                            # bank may set it.
                            nc.tensor.matmul(
                                dst,
                                slab[:, kk, j * 128:(j + 1) * 128],
                                xht[:, k, :],
                                start=(k == 0 and j % 2 == 0),
                                stop=(k == KT1 - 1),
                                skip_group_check=(j % 2 == 1),
                            )
                for j in range(FT):
                    src = pst[j // 2][:, (j % 2) * CH:(j % 2 + 1) * CH]
                    if fh == 0:
                        st = psh.tile([128, CH], BF16, tag="siluh",
                                      name=f"siluh_{j}")
                        nc.scalar.activation(st[:], src, SILU)
                        silu_tiles.append(st)
                    else:
                        nc.vector.tensor_mul(acth[:, j, :],
                                             silu_tiles[j][:], src)

            # ---- light GEMM2 (X-stationary) ----
            ps2 = [pp.tile([128, 512], dt.float32, tag="ps", name=f"psL2{i}")
                   for i in range(L2_CH)]
            for it in range(KT2):
                slab = pw2l.tile([128, H], BF16, tag="w2l")
                nc.sync.dma_start(slab[:], w2l[it, :, :])
                for ch in range(L2_CH):
                    nc.tensor.matmul(
                        ps2[ch][:], actlt[:, it, :],
                        slab[:, ch * 512:(ch + 1) * 512],
                        start=(it == 0), stop=(it == KT2 - 1),
                    )
            ylt = pyl.tile([128, H], BF16, tag="yl")
            for ch in range(L2_CH):
                if ch % 2 == 0:
                    nc.vector.tensor_copy(ylt[:, ch * 512:(ch + 1) * 512],
                                          ps2[ch][:])
                else:
                    nc.scalar.activation(ylt[:, ch * 512:(ch + 1) * 512],
                                         ps2[ch][:], COPY)
            nc.scalar.dma_start(yl[:, :H // 2], ylt[:, :H // 2])
            nc.sync.dma_start(yl[:, H // 2:], ylt[:, H // 2:])

            # ---- heavy GEMM2 (W-stationary, baseline structure) ----
            for mg in range(MT_GRP):
                pst2 = [pp.tile([128, 512], dt.float32, tag="ps",
                                name=f"psH2_{mg}_{i}")
                        for i in range(MT_G // 2)]
                slab2 = pw2h.tile([128, KT2, MW], BF16, tag="w2h")
                for lo, hi in ((0, 4), (4, 8), (8, 10), (10, KT2)):
                    nc.sync.dma_start(slab2[:, lo:hi, :],
                                      w2h[mg, :, lo:hi, :])
                for k2 in range(KT2):
                    for m in range(MT_G):
                        dst = pst2[m // 2][:, (m % 2) * CH:(m % 2 + 1) * CH]
                        nc.tensor.matmul(
                            dst,
                            slab2[:, k2, m * 128:(m + 1) * 128],
                            acth[:, k2, :],
                            start=(k2 == 0 and m % 2 == 0),
                            stop=(k2 == KT2 - 1),
                            skip_group_check=(m % 2 == 1),
                        )
                ybig = pyh.tile([128, MT_G, CH], BF16, tag="yh")
                for m in range(MT_G):
                    src = pst2[m // 2][:, (m % 2) * CH:(m % 2 + 1) * CH]
                    if m % 2 == 0:
                        nc.vector.tensor_copy(ybig[:, m, :], src)
                    else:
                        nc.scalar.activation(ybig[:, m, :], src, COPY)
                half = MT_G // 2
                nc.scalar.dma_start(yh[mg, :, :half, :], ybig[:, :half, :])
                nc.sync.dma_start(yh[mg, :, half:, :], ybig[:, half:, :])

    nc.compile()
    return nc


def _get_program():
    global _CACHED_NC
    if _CACHED_NC is None:
        _CACHED_NC = _build_program()
    return _CACHED_NC


def _route(router_logits):
    """Replicate the reference routing in numpy (fp32)."""
    lm = router_logits - router_logits.max(axis=-1, keepdims=True)
    p = np.exp(lm)
    probs = p / p.sum(axis=-1, keepdims=True)
    topi = np.argsort(-probs, axis=-1, kind="stable")[:, :TOP_K]
    topw = np.take_along_axis(probs, topi, axis=-1)
    topw = topw / topw.sum(axis=-1, keepdims=True)

    rid = topi.reshape(-1)
    rtok = np.arange(T * TOP_K) // TOP_K
    order = np.argsort(rid, kind="stable")
    counts = np.bincount(rid, minlength=E)
    offsets = np.concatenate([[0], np.cumsum(counts)[:-1]])
    return topw, rid, rtok, order, counts, offsets


def _xt_for(x, rows, rtok, ck):
    """[128, KT1, ck] transposed token slab for one expert."""
    out = np.zeros((128, KT1, ck), ml_dtypes.bfloat16)
    if len(rows):
        out[:, :, :len(rows)] = (
            x[rtok[rows]].T.reshape(KT1, 128, -1).transpose(1, 0, 2)
        ).astype(ml_dtypes.bfloat16)
    return out


def _prepare(x, w13_weight, w2_weight, expert_rows, rtok, light_ids,
             heavy_ids):
    """Per-core input maps for the hybrid program."""
    bf = ml_dtypes.bfloat16
    in_maps = []
    for c in range(N_CORES):
        gl, gh = light_ids[c], heavy_ids[c]
        # light weights
        wt = np.ascontiguousarray(w13_weight[gl].T).astype(bf)  # [H, 2I]
        gate, up = wt[:, :I], wt[:, I:]
        wa = np.concatenate(
            [gate[:, 0:512], up[:, 0:512], gate[:, 512:1024], up[:, 512:1024]],
            axis=1).reshape(KT1, 128, LA_COLS)
        wb = np.concatenate(
            [gate[:, 1024:I], up[:, 1024:I]], axis=1
        ).reshape(KT1, 128, LB_COLS)
        w2lt = np.ascontiguousarray(w2_weight[gl].T).astype(bf).reshape(
            KT2, 128, H)
        # heavy weights (baseline layouts)
        w13ht = (
            w13_weight[gh].T.astype(bf)
            .reshape(KT1 // KB1, KB1, 128, 2, I)
            .transpose(3, 0, 2, 1, 4)
        )
        w2ht = (
            w2_weight[gh].T.astype(bf).reshape(KT2, 128, MT_GRP, MW)
            .transpose(2, 1, 0, 3)
        )
        in_maps.append({
            "xl": _xt_for(x, expert_rows[gl], rtok, CL),
            "xh": _xt_for(x, expert_rows[gh], rtok, CH),
            "w13la": np.ascontiguousarray(wa),
            "w13lb": np.ascontiguousarray(wb),
            "w2l": w2lt,
            "w13h": np.ascontiguousarray(w13ht),
            "w2h": np.ascontiguousarray(w2ht),
        })
    return in_maps


def _decode(res, expert_rows, light_ids, heavy_ids, ybuf):
    """Scatter per-core outputs into ybuf[g, pos, :] (fp32)."""
    for c in range(N_CORES):
        gl, gh = light_ids[c], heavy_ids[c]
        nl = len(expert_rows[gl])
        if nl:
            ybuf[gl, :nl] = res.results[c]["yl"][:nl].astype(np.float32)
        nh = len(expert_rows[gh])
        if nh:
            ytr = (res.results[c]["yh"].transpose(0, 2, 1, 3)
                   .reshape(H, CH)).astype(np.float32)
            ybuf[gh, :nh] = ytr[:, :nh].T


# ---------------------------------------------------------------------------
# Fallback: baseline W-stationary-for-all program (handles any expert load up
# to the reference capacity via chunking).  Used only when the hybrid
# preconditions (8 experts <= 128 tokens, 8 experts <= 152) do not hold.
# ---------------------------------------------------------------------------

def _build_program_fallback(ck):
    nc = bacc.Bacc("TRN2", target_bir_lowering=False, debug=False,
                   num_devices=N_CORES)
    w13t = nc.declare_dram_parameter("w13t", [EPC, 2, KT1 // KB1, 128, KB1, I],
                                     BF16, isOutput=False)
    w2t = nc.declare_dram_parameter("w2t", [EPC, MT_GRP, 128, KT2, MW],
                                    BF16, isOutput=False)
    xt = nc.declare_dram_parameter("xt", [EPC, 128, KT1, ck], BF16,
                                   isOutput=False)
    yt = nc.declare_dram_parameter("yt", [EPC, MT_GRP, 128, MT_G, ck],
                                   BF16, isOutput=True)

    with tile.TileContext(nc) as tc:
        with (
            tc.tile_pool(name="xpool", bufs=2) as xpool,
            tc.tile_pool(name="w1pool", bufs=6) as w1pool,
            tc.tile_pool(name="w2pool", bufs=4) as w2pool,
            tc.tile_pool(name="spool", bufs=FT + 1) as spool,
            tc.tile_pool(name="apool", bufs=2 * FT) as apool,
            tc.tile_pool(name="ypool", bufs=2) as ypool,
            tc.tile_pool(name="psum", bufs=8, space="PSUM") as pspool,
        ):
            for e in range(EPC):
                xte = xpool.tile([128, KT1, ck], BF16, tag="xte")
                nc.gpsimd.dma_start(xte[:], xt[e, :, :, :])
                silu_tiles = []
                act_tiles = []
                for fh in range(2):
                    pst = [pspool.tile([128, 2 * ck], dt.float32, tag="ps",
                                       name=f"ps1_{e}_{fh}_{i}")
                           for i in range((FT + 1) // 2)]
                    for kh in range(KT1 // KB1):
                        slab = w1pool.tile([128, KB1, I], BF16, tag="w13")
                        np_pieces = KB1 if (e == 0 and fh == 0) else 2
                        step = KB1 // np_pieces
                        for pi in range(np_pieces):
                            lo = pi * step
                            nc.sync.dma_start(slab[:, lo:lo + step, :],
                                              w13t[e, fh, kh, :, lo:lo + step, :])
                        for kk in range(KB1):
                            k = kh * KB1 + kk
                            for j in range(FT):
                                dst = pst[j // 2][:,
                                                  (j % 2) * ck:(j % 2 + 1) * ck]
                                nc.tensor.matmul(
                                    dst,
                                    slab[:, kk, j * 128:(j + 1) * 128],
                                    xte[:, k, :],
                                    start=(k == 0 and j % 2 == 0),
                                    stop=(k == KT1 - 1),
                                    skip_group_check=(j % 2 == 1),
                                )
                    for j in range(FT):
                        src = pst[j // 2][:, (j % 2) * ck:(j % 2 + 1) * ck]
                        if fh == 0:
                            st = spool.tile([128, ck], BF16, tag="silu",
                                            name=f"silu_{e}_{j}")
                            nc.scalar.activation(st[:], src, SILU)
                            silu_tiles.append(st)
                        else:
                            at = apool.tile([128, ck], BF16, tag="act",
                                            name=f"act_{e}_{j}")
                            nc.vector.tensor_mul(at[:], silu_tiles[j][:], src)
                            act_tiles.append(at)
                for mg in range(MT_GRP):
                    pst2 = [pspool.tile([128, 2 * ck], dt.float32, tag="ps",
                                        name=f"ps2_{e}_{mg}_{i}")
                            for i in range(MT_G // 2)]
                    slab2 = w2pool.tile([128, KT2, MW], BF16, tag="w2")
                    for lo, hi in ((0, 4), (4, 8), (8, 10), (10, KT2)):
                        nc.sync.dma_start(slab2[:, lo:hi, :],
                                          w2t[e, mg, :, lo:hi, :])
                    for k2 in range(KT2):
                        for m in range(MT_G):
                            dst = pst2[m // 2][:, (m % 2) * ck:(m % 2 + 1) * ck]
                            nc.tensor.matmul(
                                dst,
                                slab2[:, k2, m * 128:(m + 1) * 128],
                                act_tiles[k2][:],
                                start=(k2 == 0 and m % 2 == 0),
                                stop=(k2 == KT2 - 1),
                                skip_group_check=(m % 2 == 1),
                            )
                    ybig = ypool.tile([128, MT_G, ck], BF16, tag="y")
                    for m in range(MT_G):
                        src = pst2[m // 2][:, (m % 2) * ck:(m % 2 + 1) * ck]
                        nc.vector.tensor_copy(ybig[:, m, :], src)
                    half = MT_G // 2
                    nc.sync.dma_start(yt[e, mg, :, :half, :],
                                      ybig[:, :half, :])
                    nc.scalar.dma_start(yt[e, mg, :, half:, :],
                                        ybig[:, half:, :])

    nc.compile()
    return nc


def _run_fallback(x, w13_weight, w2_weight, expert_rows, rtok, eff):
    ck = max(176, -(-eff // 8) * 8)
    if ck not in _CACHED_FB:
        _CACHED_FB[ck] = _build_program_fallback(ck)
    nc = _CACHED_FB[ck]
    bf = ml_dtypes.bfloat16
    n_chunks = max(1, -(-eff // ck))
    ybuf = np.zeros((E, eff, H), np.float32)
    w13t_cores, w2t_cores = [], []
    for c in range(N_CORES):
        a = np.empty((EPC, 2, KT1 // KB1, 128, KB1, I), bf)
        b = np.empty((EPC, MT_GRP, 128, KT2, MW), bf)
        for el in range(EPC):
            g = c * EPC + el
            a[el] = (w13_weight[g].T.reshape(KT1 // KB1, KB1, 128, 2, I)
                     .transpose(3, 0, 2, 1, 4))
            b[el] = (w2_weight[g].T.reshape(KT2, 128, MT_GRP, MW)
                     .transpose(2, 1, 0, 3))
        w13t_cores.append(a)
        w2t_cores.append(b)
    for chunk in range(n_chunks):
        in_maps = []
        for c in range(N_CORES):
            xt_c = np.zeros((EPC, 128, KT1, ck), bf)
            for el in range(EPC):
                g = c * EPC + el
                rows = expert_rows[g][chunk * ck:(chunk + 1) * ck]
                xt_c[el] = _xt_for(x, rows, rtok, ck)
            in_maps.append(
                {"w13t": w13t_cores[c], "w2t": w2t_cores[c], "xt": xt_c}
            )
        res = run_bass_kernel_spmd(nc, in_maps, list(range(N_CORES)))
        for c in range(N_CORES):
            yt_c = res.results[c]["yt"]
            for el in range(EPC):
                g = c * EPC + el
                n = len(expert_rows[g][chunk * ck:(chunk + 1) * ck])
                if n:
                    ytr = (yt_c[el].transpose(0, 2, 1, 3)
                           .reshape(H, ck)).astype(np.float32)
                    ybuf[g, chunk * ck:chunk * ck + n] = ytr[:, :n].T
    return ybuf


def kernel(x, router_logits, w13_weight, w2_weight):
    x = np.asarray(x, dtype=np.float32)
    router_logits = np.asarray(router_logits, dtype=np.float32)
    w13_weight = np.asarray(w13_weight, dtype=np.float32)
    w2_weight = np.asarray(w2_weight, dtype=np.float32)
    assert x.shape == (T, H) and router_logits.shape == (T, E)
    assert w13_weight.shape == (E, TWO_I, H) and w2_weight.shape == (E, H, I)

    topw, rid, rtok, order, counts, offsets = _route(router_logits)
    # reference capacity: rows with in-expert position >= 512 are dropped
    CAP = 512
    eff = int(min(counts.max(), CAP))
    expert_rows = [
        order[offsets[g]:offsets[g] + min(int(counts[g]), CAP)]
        for g in range(E)
    ]

    by_load = np.argsort(-counts, kind="stable")
    heavy_ids = [int(g) for g in by_load[:N_CORES]]
    light_ids = [int(g) for g in by_load[N_CORES:]]
    hybrid_ok = (
        counts[heavy_ids].max() <= CH and counts[light_ids].max() <= CL
    )

    if hybrid_ok:
        nc = _get_program()
        in_maps = _prepare(x, w13_weight, w2_weight, expert_rows, rtok,
                           light_ids, heavy_ids)
        ybuf = np.zeros((E, eff, H), np.float32)

        def _run():
            res = run_bass_kernel_spmd(nc, in_maps, list(range(N_CORES)))
            _decode(res, expert_rows, light_ids, heavy_ids, ybuf)

        def _spot_ok():
            # one token per expert vs numpy fp32: catches rare flaky-device
            # corruption (bf16 path error is ~5e-3, far under the gate)
            for g in range(E):
                rows = expert_rows[g]
                if not len(rows):
                    continue
                tok = rtok[rows[0]]
                h = x[tok] @ w13_weight[g].T
                act = h[:I] / (1.0 + np.exp(-h[:I])) * h[I:]
                yref = act @ w2_weight[g].T
                got = ybuf[g, 0]
                if np.linalg.norm(got - yref) > 0.05 * np.linalg.norm(yref):
                    return False
            return True

        _run()
        if not _spot_ok():
            _run()  # one retry on a flaky device result
    else:
        ybuf = _run_fallback(x, w13_weight, w2_weight, expert_rows, rtok, eff)

    # ---- combine: gather rows back, weight by router probs ----
    pos = np.empty(T * TOP_K, np.int64)
    for g in range(E):
        pos[order[offsets[g]:offsets[g] + counts[g]]] = np.arange(counts[g])
    valid = (pos < CAP).astype(np.float32)
    posc = np.minimum(pos, eff - 1)
    yrows = ybuf[rid, posc] * valid[:, None]  # [T*K, H]
    out = np.einsum(
        "tkh,tk->th", yrows.reshape(T, TOP_K, H), topw.astype(np.float32)
    )
    return out.astype(np.float32)


# revision 8
# speedup vs baseline: 1.2510x; 1.2510x over previous
"""EPMoE (top-2, 16 experts) forward on 8 Trainium2 NeuronCores.

Strategy (expert parallel, hybrid dataflow):
  - Host: router softmax/top-2/renorm, token->expert dispatch (stable order,
    matching the reference), weight re-layout into slab-contiguous form,
    final weighted combine.
  - Device (per core, 2 experts): one LIGHT expert (<=128 tokens) computed
    with the token tile as the matmul stationary operand (weights stream as
    the moving operand, cutting LDWEIGHTS overhead ~10x and producing
    token-major outputs), and one HEAVY expert (<=152 tokens) computed with
    the classic weight-stationary grouped GEMM.  Tensor-engine phases are
    ordered L-G1 -> H-G1 -> L-G2 -> H-G2 so activation drains (silu / mul /
    act transpose) always hide under another expert's matmuls, and the
    weight DMA stream (the memory-roofline term: ~34.6 MB/core in bf16)
    matches consumption order exactly.

The reference's simulated fp8 quantization (amax scaling + clip, no rounding)
cancels exactly: (x/sa) @ (w/sw)^T * sa*sw == x @ w^T, and the +-448 clip
never binds for amax-scaled values.  So the kernel computes the plain MoE
forward in bf16 (real fp8 rounding would blow the 2e-2 error budget).
"""

import ml_dtypes
import numpy as np

import concourse.bass as bass
import concourse.bacc as bacc
import concourse.masks as masks
import concourse.mybir as mybir
import concourse.tile as tile
from concourse.bass_utils import run_bass_kernel_spmd

dt = mybir.dt

# Problem shape (hardcoded per spec)
T, H, I, E, TOP_K = 1024, 2048, 1408, 16, 2
TWO_I = 2 * I
N_CORES = 8
EPC = E // N_CORES

KT1 = H // 128      # 16 contraction tiles for GEMM1
FT = I // 128       # 11 feature tiles per gate/up half
KT2 = I // 128      # 11 contraction tiles for GEMM2

# Light (X-stationary) expert: token count must be <= CL.
CL = 128
# GEMM1 f-columns are laid out as interleaved (gate, up) 512-chunks so each
# psum bank pair (g_j, u_j) drains to silu*mul independently.
#   pass A: [g 0:512 | u 0:512 | g 512:1024 | u 512:1024]   (2048 cols)
#   pass B: [g 1024:1408 | u 1024:1408]                     (768 cols)
LA_COLS = 2048
LB_COLS = 768
L2_CH = 4           # GEMM2 output 512-chunks (4 * 512 = 2048 = H)

# Heavy (W-stationary) expert: token count must be <= CH.
CH = 152
KB1 = 4             # k-tiles per heavy GEMM1 weight slab (one DMA)
MT_GRP = 4          # heavy GEMM2 m-groups (small groups shorten the tail)
MT_G = H // 128 // MT_GRP  # 4 output tiles per m-group
MW = MT_G * 128     # 512
# fallback program keeps the original 2-group split
FB_MT_GRP = 2
FB_MT_G = H // 128 // FB_MT_GRP
FB_MW = FB_MT_G * 128

_CACHED_NC = None
_CACHED_FB = {}

F32R = dt.float32r
BF16 = dt.bfloat16
SILU = mybir.ActivationFunctionType.Silu
COPY = mybir.ActivationFunctionType.Copy


def _build_program():
    """One SPMD program: per core, 1 light + 1 heavy expert MoE FFN.

    DRAM layouts (host pre-arranged, all bf16):
      xl   [128, KT1, CL]       x^T of the light expert's tokens
      xh   [128, KT1, CH]       x^T of the heavy expert's tokens
      w13la[KT1, 128, LA_COLS]  light w13^T cols [g0|u0|g1|u1] (512-chunks)
      w13lb[KT1, 128, LB_COLS]  light w13^T cols [g2|u2] (384-chunks)
      w2l  [KT2, 128, H]        light w2^T
      w13h [2, KT1//KB1, 128, KB1, I]  heavy w13^T, gate/up halves
      w2h  [MT_GRP, 128, KT2, MW]      heavy w2^T
      yl   [128, H]   (out)     light y, token-major
      yh   [MT_GRP, 128, MT_G, CH] (out)  heavy y^T
    """
    nc = bacc.Bacc("TRN2", target_bir_lowering=False, debug=False,
                   num_devices=N_CORES)

    xl = nc.declare_dram_parameter("xl", [128, KT1, CL], BF16, isOutput=False)
    xh = nc.declare_dram_parameter("xh", [128, KT1, CH], BF16, isOutput=False)
    w13la = nc.declare_dram_parameter("w13la", [KT1 // 2, 128, 2, LA_COLS],
                                      BF16, isOutput=False)
    w13lb = nc.declare_dram_parameter("w13lb", [KT1 // 4, 128, 4, LB_COLS],
                                      BF16, isOutput=False)
    w2l = nc.declare_dram_parameter("w2l", [KT2, 128, H], BF16, isOutput=False)
    w13h = nc.declare_dram_parameter("w13h", [2, KT1 // KB1, 128, KB1, I],
                                     BF16, isOutput=False)
    w2h = nc.declare_dram_parameter("w2h", [MT_GRP, 128, KT2, MW], BF16,
                                    isOutput=False)
    yl = nc.declare_dram_parameter("yl", [128, H], BF16, isOutput=True)
    yh = nc.declare_dram_parameter("yh", [MT_GRP, 128, MT_G, CH], BF16,
                                   isOutput=True)

    with tile.TileContext(nc) as tc:
        with (
            tc.tile_pool(name="pxl", bufs=1) as pxl,
            tc.tile_pool(name="pxh", bufs=1) as pxh,
            tc.tile_pool(name="pident", bufs=1) as pident,
            tc.tile_pool(name="pwla", bufs=4) as pwla,
            tc.tile_pool(name="pwlb", bufs=2) as pwlb,
            tc.tile_pool(name="pw13h", bufs=3) as pw13h,
            tc.tile_pool(name="pw2l", bufs=5) as pw2l,
            tc.tile_pool(name="pw2h", bufs=3) as pw2h,
            tc.tile_pool(name="psl", bufs=1) as psl,
            tc.tile_pool(name="pactl", bufs=1) as pactl,
            tc.tile_pool(name="pactlt", bufs=1) as pactlt,
            tc.tile_pool(name="psh", bufs=FT + 1) as psh,
            tc.tile_pool(name="pacth", bufs=1) as pacth,
            tc.tile_pool(name="pyl", bufs=1) as pyl,
            tc.tile_pool(name="pyh", bufs=2) as pyh,
            tc.tile_pool(name="psum", bufs=8, space="PSUM") as pp,
        ):
            # x loads on the scalar HWDGE queue so they run concurrently
            # with the weight stream on the sync queue from t=0.
            xlt = pxl.tile([128, KT1, CL], BF16, tag="xl")
            xht = pxh.tile([128, KT1, CH], BF16, tag="xh")
            nc.scalar.dma_start(xlt[:, :2, :], xl[:, :2, :])
            nc.scalar.dma_start(xlt[:, 2:8, :], xl[:, 2:8, :])
            nc.scalar.dma_start(xlt[:, 8:, :], xl[:, 8:, :])
            nc.scalar.dma_start(xht[:], xh[:])
            ident = pident.tile([128, 128], dt.float32, tag="ident")
            masks.make_identity(nc, ident[:])

            # ---- light GEMM1 pass A: psum chunks [g0 | u0 | g1 | u1] ----
            psA = [pp.tile([128, 512], dt.float32, tag="ps", name=f"psA{i}")
                   for i in range(4)]
            for kh2 in range(KT1 // 2):
                slab = pwla.tile([128, 2, LA_COLS], BF16, tag="wla")
                # finest pieces on the first slab so the PE starts early
                if kh2 == 0:
                    for k in range(2):
                        for pi in range(2):
                            lo = pi * (LA_COLS // 2)
                            nc.sync.dma_start(
                                slab[:, k, lo:lo + LA_COLS // 2],
                                w13la[kh2, :, k, lo:lo + LA_COLS // 2])
                else:
                    for k in range(2):
                        nc.sync.dma_start(slab[:, k, :], w13la[kh2, :, k, :])
                for k in range(2):
                    kh = kh2 * 2 + k
                    for ch in range(4):
                        nc.tensor.matmul(
                            psA[ch][:], xlt[:, kh, :],
                            slab[:, k, ch * 512:(ch + 1) * 512],
                            start=(kh == 0), stop=(kh == KT1 - 1),
                        )
            # drain A while pass B computes: act[:, 0:1024]
            slt = psl.tile([128, I], BF16, tag="sl")
            actl = pactl.tile([128, I], dt.float32, tag="actl")
            for pr in range(2):
                nc.scalar.activation(slt[:, pr * 512:(pr + 1) * 512],
                                     psA[2 * pr][:], SILU)
                nc.vector.tensor_mul(actl[:, pr * 512:(pr + 1) * 512],
                                     slt[:, pr * 512:(pr + 1) * 512],
                                     psA[2 * pr + 1][:])

            # ---- light GEMM1 pass B: psum chunks [g2 | u2] (384 wide) ----
            psB = [pp.tile([128, 512], dt.float32, tag="ps", name=f"psB{i}")
                   for i in range(2)]
            for kh4 in range(KT1 // 4):
                slab = pwlb.tile([128, 4, LB_COLS], BF16, tag="wlb")
                nc.sync.dma_start(slab[:], w13lb[kh4, :, :, :])
                for k in range(4):
                    kh = kh4 * 4 + k
                    for ch in range(2):
                        nc.tensor.matmul(
                            psB[ch][:, :384], xlt[:, kh, :],
                            slab[:, k, ch * 384:(ch + 1) * 384],
                            start=(kh == 0), stop=(kh == KT1 - 1),
                        )
            nc.scalar.activation(slt[:, 1024:I], psB[0][:, :384], SILU)
            nc.vector.tensor_mul(actl[:, 1024:I], slt[:, 1024:I],
                                 psB[1][:, :384])

            # ---- heavy GEMM1 gate half (W-stationary) ----
            silu_tiles = []
            acth = pacth.tile([128, KT2, CH], BF16, tag="acth")
            pst_g = [pp.tile([128, 512], dt.float32, tag="ps",
                             name=f"psH0_{i}")
                     for i in range((FT + 1) // 2)]
            for kh in range(KT1 // KB1):
                slab = pw13h.tile([128, KB1, I], BF16, tag="w13h")
                for pi in range(2):
                    lo = pi * (KB1 // 2)
                    nc.sync.dma_start(slab[:, lo:lo + KB1 // 2, :],
                                      w13h[0, kh, :, lo:lo + KB1 // 2, :])
                for kk in range(KB1):
                    k = kh * KB1 + kk
                    for j in range(FT):
                        dst = pst_g[j // 2][:, (j % 2) * CH:(j % 2 + 1) * CH]
                        # start=True clears has_written for the WHOLE bank:
                        # only the first group packed into each bank sets it.
                        nc.tensor.matmul(
                            dst,
                            slab[:, kk, j * 128:(j + 1) * 128],
                            xht[:, k, :],
                            start=(k == 0 and j % 2 == 0),
                            stop=(k == KT1 - 1),
                            skip_group_check=(j % 2 == 1),
                        )
            for j in range(FT):
                src = pst_g[j // 2][:, (j % 2) * CH:(j % 2 + 1) * CH]
                st = psh.tile([128, CH], BF16, tag="siluh", name=f"siluh_{j}")
                nc.scalar.activation(st[:], src, SILU)
                silu_tiles.append(st)

            # light act transpose on the PE (gate silus drain pst_g banks
            # meanwhile; xbar DMA transpose would throttle the HBM stream)
            actlt = pactlt.tile([128, KT2, 128], BF16, tag="actlt")
            for j in range(KT2):
                ptr = pp.tile([128, 512], dt.float32, tag="ps",
                              name=f"ptr{j}")
                nc.tensor.matmul(ptr[:, :128], actl[:, j * 128:(j + 1) * 128],
                                 ident[:], is_transpose=True)
                nc.vector.tensor_copy(actlt[:, j, :], ptr[:, :128])

            # ---- heavy GEMM1 up half ----
            pst_u = [pp.tile([128, 512], dt.float32, tag="ps",
                             name=f"psH1_{i}")
                     for i in range((FT + 1) // 2)]
            for kh in range(KT1 // KB1):
                slab = pw13h.tile([128, KB1, I], BF16, tag="w13h")
                for pi in range(2):
                    lo = pi * (KB1 // 2)
                    nc.sync.dma_start(slab[:, lo:lo + KB1 // 2, :],
                                      w13h[1, kh, :, lo:lo + KB1 // 2, :])
                for kk in range(KB1):
                    k = kh * KB1 + kk
                    for j in range(FT):
                        dst = pst_u[j // 2][:, (j % 2) * CH:(j % 2 + 1) * CH]
                        nc.tensor.matmul(
                            dst,
                            slab[:, kk, j * 128:(j + 1) * 128],
                            xht[:, k, :],
                            start=(k == 0 and j % 2 == 0),
                            stop=(k == KT1 - 1),
                            skip_group_check=(j % 2 == 1),
                        )
            for j in range(FT):
                src = pst_u[j // 2][:, (j % 2) * CH:(j % 2 + 1) * CH]
                nc.vector.tensor_mul(acth[:, j, :], silu_tiles[j][:], src)

            # ---- light GEMM2 (X-stationary) ----
            ps2 = [pp.tile([128, 512], dt.float32, tag="ps", name=f"psL2{i}")
                   for i in range(L2_CH)]
            for it in range(KT2):
                slab = pw2l.tile([128, H], BF16, tag="w2l")
                nc.sync.dma_start(slab[:], w2l[it, :, :])
                for ch in range(L2_CH):
                    nc.tensor.matmul(
                        ps2[ch][:], actlt[:, it, :],
                        slab[:, ch * 512:(ch + 1) * 512],
                        start=(it == 0), stop=(it == KT2 - 1),
                    )
            ylt = pyl.tile([128, H], BF16, tag="yl")
            for ch in range(L2_CH):
                if ch % 2 == 0:
                    nc.vector.tensor_copy(ylt[:, ch * 512:(ch + 1) * 512],
                                          ps2[ch][:])
                else:
                    nc.scalar.activation(ylt[:, ch * 512:(ch + 1) * 512],
                                         ps2[ch][:], COPY)
            nc.scalar.dma_start(yl[:, :H // 2], ylt[:, :H // 2])
            nc.sync.dma_start(yl[:, H // 2:], ylt[:, H // 2:])

            # ---- heavy GEMM2 (W-stationary, 4 small m-groups) ----
            for mg in range(MT_GRP):
                pst2 = [pp.tile([128, 512], dt.float32, tag="ps",
                                name=f"psH2_{mg}_{i}")
                        for i in range(MT_G // 2)]
                slab2 = pw2h.tile([128, KT2, MW], BF16, tag="w2h")
                for lo, hi in ((0, 6), (6, KT2)):
                    nc.sync.dma_start(slab2[:, lo:hi, :],
                                      w2h[mg, :, lo:hi, :])
                for k2 in range(KT2):
                    for m in range(MT_G):
                        dst = pst2[m // 2][:, (m % 2) * CH:(m % 2 + 1) * CH]
                        nc.tensor.matmul(
                            dst,
                            slab2[:, k2, m * 128:(m + 1) * 128],
                            acth[:, k2, :],
                            start=(k2 == 0 and m % 2 == 0),
                            stop=(k2 == KT2 - 1),
                            skip_group_check=(m % 2 == 1),
                        )
                ybig = pyh.tile([128, MT_G, CH], BF16, tag="yh")
                for m in range(MT_G):
                    src = pst2[m // 2][:, (m % 2) * CH:(m % 2 + 1) * CH]
                    if m % 2 == 0:
                        nc.vector.tensor_copy(ybig[:, m, :], src)
                    else:
                        nc.scalar.activation(ybig[:, m, :], src, COPY)
                half = MT_G // 2
                nc.scalar.dma_start(yh[mg, :, :half, :], ybig[:, :half, :])
                nc.sync.dma_start(yh[mg, :, half:, :], ybig[:, half:, :])

    nc.compile()
    return nc


def _get_program():
    global _CACHED_NC
    if _CACHED_NC is None:
        _CACHED_NC = _build_program()
    return _CACHED_NC


def _route(router_logits):
    """Replicate the reference routing in numpy (fp32)."""
    lm = router_logits - router_logits.max(axis=-1, keepdims=True)
    p = np.exp(lm)
    probs = p / p.sum(axis=-1, keepdims=True)
    topi = np.argsort(-probs, axis=-1, kind="stable")[:, :TOP_K]
    topw = np.take_along_axis(probs, topi, axis=-1)
    topw = topw / topw.sum(axis=-1, keepdims=True)

    rid = topi.reshape(-1)
    rtok = np.arange(T * TOP_K) // TOP_K
    order = np.argsort(rid, kind="stable")
    counts = np.bincount(rid, minlength=E)
    offsets = np.concatenate([[0], np.cumsum(counts)[:-1]])
    return topw, rid, rtok, order, counts, offsets


def _xt_for(x, rows, rtok, ck):
    """[128, KT1, ck] transposed token slab for one expert."""
    out = np.zeros((128, KT1, ck), ml_dtypes.bfloat16)
    if len(rows):
        out[:, :, :len(rows)] = (
            x[rtok[rows]].T.reshape(KT1, 128, -1).transpose(1, 0, 2)
        ).astype(ml_dtypes.bfloat16)
    return out


def _prepare(x, w13_weight, w2_weight, expert_rows, rtok, light_ids,
             heavy_ids):
    """Per-core input maps for the hybrid program."""
    bf = ml_dtypes.bfloat16
    in_maps = []
    for c in range(N_CORES):
        gl, gh = light_ids[c], heavy_ids[c]
        # light weights
        wt = np.ascontiguousarray(w13_weight[gl].T).astype(bf)  # [H, 2I]
        gate, up = wt[:, :I], wt[:, I:]
        wa = np.concatenate(
            [gate[:, 0:512], up[:, 0:512], gate[:, 512:1024], up[:, 512:1024]],
            axis=1).reshape(KT1 // 2, 2, 128, LA_COLS).transpose(0, 2, 1, 3)
        wb = np.concatenate(
            [gate[:, 1024:I], up[:, 1024:I]], axis=1
        ).reshape(KT1 // 4, 4, 128, LB_COLS).transpose(0, 2, 1, 3)
        w2lt = np.ascontiguousarray(w2_weight[gl].T).astype(bf).reshape(
            KT2, 128, H)
        # heavy weights (baseline layouts)
        w13ht = (
            w13_weight[gh].T.astype(bf)
            .reshape(KT1 // KB1, KB1, 128, 2, I)
            .transpose(3, 0, 2, 1, 4)
        )
        w2ht = (
            w2_weight[gh].T.astype(bf).reshape(KT2, 128, MT_GRP, MW)
            .transpose(2, 1, 0, 3)
        )
        in_maps.append({
            "xl": _xt_for(x, expert_rows[gl], rtok, CL),
            "xh": _xt_for(x, expert_rows[gh], rtok, CH),
            "w13la": np.ascontiguousarray(wa),
            "w13lb": np.ascontiguousarray(wb),
            "w2l": w2lt,
            "w13h": np.ascontiguousarray(w13ht),
            "w2h": np.ascontiguousarray(w2ht),
        })
    return in_maps


def _decode(res, expert_rows, light_ids, heavy_ids, ybuf):
    """Scatter per-core outputs into ybuf[g, pos, :] (fp32)."""
    for c in range(N_CORES):
        gl, gh = light_ids[c], heavy_ids[c]
        nl = len(expert_rows[gl])
        if nl:
            ybuf[gl, :nl] = res.results[c]["yl"][:nl].astype(np.float32)
        nh = len(expert_rows[gh])
        if nh:
            ytr = (res.results[c]["yh"].transpose(0, 2, 1, 3)
                   .reshape(H, CH)).astype(np.float32)
            ybuf[gh, :nh] = ytr[:, :nh].T


# ---------------------------------------------------------------------------
# Fallback: baseline W-stationary-for-all program (handles any expert load up
# to the reference capacity via chunking).  Used only when the hybrid
# preconditions (8 experts <= 128 tokens, 8 experts <= 152) do not hold.
# ---------------------------------------------------------------------------

def _build_program_fallback(ck):
    MT_GRP, MT_G, MW = FB_MT_GRP, FB_MT_G, FB_MW
    nc = bacc.Bacc("TRN2", target_bir_lowering=False, debug=False,
                   num_devices=N_CORES)
    w13t = nc.declare_dram_parameter("w13t", [EPC, 2, KT1 // KB1, 128, KB1, I],
                                     BF16, isOutput=False)
    w2t = nc.declare_dram_parameter("w2t", [EPC, MT_GRP, 128, KT2, MW],
                                    BF16, isOutput=False)
    xt = nc.declare_dram_parameter("xt", [EPC, 128, KT1, ck], BF16,
                                   isOutput=False)
    yt = nc.declare_dram_parameter("yt", [EPC, MT_GRP, 128, MT_G, ck],
                                   BF16, isOutput=True)

    with tile.TileContext(nc) as tc:
        with (
            tc.tile_pool(name="xpool", bufs=2) as xpool,
            tc.tile_pool(name="w1pool", bufs=6) as w1pool,
            tc.tile_pool(name="w2pool", bufs=4) as w2pool,
            tc.tile_pool(name="spool", bufs=FT + 1) as spool,
            tc.tile_pool(name="apool", bufs=2 * FT) as apool,
            tc.tile_pool(name="ypool", bufs=2) as ypool,
            tc.tile_pool(name="psum", bufs=8, space="PSUM") as pspool,
        ):
            for e in range(EPC):
                xte = xpool.tile([128, KT1, ck], BF16, tag="xte")
                nc.gpsimd.dma_start(xte[:], xt[e, :, :, :])
                silu_tiles = []
                act_tiles = []
                for fh in range(2):
                    pst = [pspool.tile([128, 2 * ck], dt.float32, tag="ps",
                                       name=f"ps1_{e}_{fh}_{i}")
                           for i in range((FT + 1) // 2)]
                    for kh in range(KT1 // KB1):
                        slab = w1pool.tile([128, KB1, I], BF16, tag="w13")
                        np_pieces = KB1 if (e == 0 and fh == 0) else 2
                        step = KB1 // np_pieces
                        for pi in range(np_pieces):
                            lo = pi * step
                            nc.sync.dma_start(slab[:, lo:lo + step, :],
                                              w13t[e, fh, kh, :, lo:lo + step, :])
                        for kk in range(KB1):
                            k = kh * KB1 + kk
                            for j in range(FT):
                                dst = pst[j // 2][:,
                                                  (j % 2) * ck:(j % 2 + 1) * ck]
                                nc.tensor.matmul(
                                    dst,
                                    slab[:, kk, j * 128:(j + 1) * 128],
                                    xte[:, k, :],
                                    start=(k == 0 and j % 2 == 0),
                                    stop=(k == KT1 - 1),
                                    skip_group_check=(j % 2 == 1),
                                )
                    for j in range(FT):
                        src = pst[j // 2][:, (j % 2) * ck:(j % 2 + 1) * ck]
                        if fh == 0:
                            st = spool.tile([128, ck], BF16, tag="silu",
                                            name=f"silu_{e}_{j}")
                            nc.scalar.activation(st[:], src, SILU)
                            silu_tiles.append(st)
                        else:
                            at = apool.tile([128, ck], BF16, tag="act",
                                            name=f"act_{e}_{j}")
                            nc.vector.tensor_mul(at[:], silu_tiles[j][:], src)
                            act_tiles.append(at)
                for mg in range(MT_GRP):
                    pst2 = [pspool.tile([128, 2 * ck], dt.float32, tag="ps",
                                        name=f"ps2_{e}_{mg}_{i}")
                            for i in range(MT_G // 2)]
                    slab2 = w2pool.tile([128, KT2, MW], BF16, tag="w2")
                    for lo, hi in ((0, 4), (4, 8), (8, 10), (10, KT2)):
                        nc.sync.dma_start(slab2[:, lo:hi, :],
                                          w2t[e, mg, :, lo:hi, :])
                    for k2 in range(KT2):
                        for m in range(MT_G):
                            dst = pst2[m // 2][:, (m % 2) * ck:(m % 2 + 1) * ck]
                            nc.tensor.matmul(
                                dst,
                                slab2[:, k2, m * 128:(m + 1) * 128],
                                act_tiles[k2][:],
                                start=(k2 == 0 and m % 2 == 0),
                                stop=(k2 == KT2 - 1),
                                skip_group_check=(m % 2 == 1),
                            )
                    ybig = ypool.tile([128, MT_G, ck], BF16, tag="y")
                    for m in range(MT_G):
                        src = pst2[m // 2][:, (m % 2) * ck:(m % 2 + 1) * ck]
                        nc.vector.tensor_copy(ybig[:, m, :], src)
                    half = MT_G // 2
                    nc.sync.dma_start(yt[e, mg, :, :half, :],
                                      ybig[:, :half, :])
                    nc.scalar.dma_start(yt[e, mg, :, half:, :],
                                        ybig[:, half:, :])

    nc.compile()
    return nc


def _run_fallback(x, w13_weight, w2_weight, expert_rows, rtok, eff):
    ck = max(176, -(-eff // 8) * 8)
    if ck not in _CACHED_FB:
        _CACHED_FB[ck] = _build_program_fallback(ck)
    nc = _CACHED_FB[ck]
    bf = ml_dtypes.bfloat16
    n_chunks = max(1, -(-eff // ck))
    ybuf = np.zeros((E, eff, H), np.float32)
    w13t_cores, w2t_cores = [], []
    for c in range(N_CORES):
        a = np.empty((EPC, 2, KT1 // KB1, 128, KB1, I), bf)
        b = np.empty((EPC, FB_MT_GRP, 128, KT2, FB_MW), bf)
        for el in range(EPC):
            g = c * EPC + el
            a[el] = (w13_weight[g].T.reshape(KT1 // KB1, KB1, 128, 2, I)
                     .transpose(3, 0, 2, 1, 4))
            b[el] = (w2_weight[g].T.reshape(KT2, 128, FB_MT_GRP, FB_MW)
                     .transpose(2, 1, 0, 3))
        w13t_cores.append(a)
        w2t_cores.append(b)
    for chunk in range(n_chunks):
        in_maps = []
        for c in range(N_CORES):
            xt_c = np.zeros((EPC, 128, KT1, ck), bf)
            for el in range(EPC):
                g = c * EPC + el
                rows = expert_rows[g][chunk * ck:(chunk + 1) * ck]
                xt_c[el] = _xt_for(x, rows, rtok, ck)
            in_maps.append(
                {"w13t": w13t_cores[c], "w2t": w2t_cores[c], "xt": xt_c}
            )
        res = run_bass_kernel_spmd(nc, in_maps, list(range(N_CORES)))
        for c in range(N_CORES):
            yt_c = res.results[c]["yt"]
            for el in range(EPC):
                g = c * EPC + el
                n = len(expert_rows[g][chunk * ck:(chunk + 1) * ck])
                if n:
                    ytr = (yt_c[el].transpose(0, 2, 1, 3)
                           .reshape(H, ck)).astype(np.float32)
                    ybuf[g, chunk * ck:chunk * ck + n] = ytr[:, :n].T
    return ybuf


def kernel(x, router_logits, w13_weight, w2_weight):
    x = np.asarray(x, dtype=np.float32)
    router_logits = np.asarray(router_logits, dtype=np.float32)
    w13_weight = np.asarray(w13_weight, dtype=np.float32)
    w2_weight = np.asarray(w2_weight, dtype=np.float32)
    assert x.shape == (T, H) and router_logits.shape == (T, E)
    assert w13_weight.shape == (E, TWO_I, H) and w2_weight.shape == (E, H, I)

    topw, rid, rtok, order, counts, offsets = _route(router_logits)
    # reference capacity: rows with in-expert position >= 512 are dropped
    CAP = 512
    eff = int(min(counts.max(), CAP))
    expert_rows = [
        order[offsets[g]:offsets[g] + min(int(counts[g]), CAP)]
        for g in range(E)
    ]

    by_load = np.argsort(-counts, kind="stable")
    heavy_ids = [int(g) for g in by_load[:N_CORES]]
    light_ids = [int(g) for g in by_load[N_CORES:]]
    hybrid_ok = (
        counts[heavy_ids].max() <= CH and counts[light_ids].max() <= CL
    )

    if hybrid_ok:
        nc = _get_program()
        in_maps = _prepare(x, w13_weight, w2_weight, expert_rows, rtok,
                           light_ids, heavy_ids)
        ybuf = np.zeros((E, eff, H), np.float32)

        def _run():
            res = run_bass_kernel_spmd(nc, in_maps, list(range(N_CORES)))
            _decode(res, expert_rows, light_ids, heavy_ids, ybuf)

        def _spot_ok():
            # one token per expert vs numpy fp32: catches rare flaky-device
            # corruption (bf16 path error is ~5e-3, far under the gate)
            for g in range(E):
                rows = expert_rows[g]
                if not len(rows):
                    continue
                tok = rtok[rows[0]]
                h = x[tok] @ w13_weight[g].T
                act = h[:I] / (1.0 + np.exp(-h[:I])) * h[I:]
                yref = act @ w2_weight[g].T
                got = ybuf[g, 0]
                if np.linalg.norm(got - yref) > 0.05 * np.linalg.norm(yref):
                    return False
            return True

        _run()
        if not _spot_ok():
            _run()  # one retry on a flaky device result
    else:
        ybuf = _run_fallback(x, w13_weight, w2_weight, expert_rows, rtok, eff)

    # ---- combine: gather rows back, weight by router probs ----
    pos = np.empty(T * TOP_K, np.int64)
    for g in range(E):
        pos[order[offsets[g]:offsets[g] + counts[g]]] = np.arange(counts[g])
    valid = (pos < CAP).astype(np.float32)
    posc = np.minimum(pos, eff - 1)
    yrows = ybuf[rid, posc] * valid[:, None]  # [T*K, H]
    out = np.einsum(
        "tkh,tk->th", yrows.reshape(T, TOP_K, H), topw.astype(np.float32)
    )
    return out.astype(np.float32)
